# revision 11
# baseline (speedup 1.0000x reference)
"""Two-layer GAT (gnn_message_passing) on 8 Trainium2 NeuronCores.

Sharding: nodes are split into 8 contiguous shards of 1024 (one per core);
each core owns every edge whose destination lies in its shard.  The halo
exchange is an on-device AllGather of the node feature table
[h | alpha_src | alpha_dst]; cores resolve incident edges with
InstDMAGatherAnt (swdge dma_gather: up to 1024 arbitrary table rows per
instruction, ~125us nearly independent of payload width).

Environment cost model (measured): ~50-80us fixed per instruction on every
engine queue; dma_gather ~125-145us for <=1024 rows of any width;
AllGather ~110us + bytes/4.5GB/s.  The kernel therefore minimizes
instruction count and balances the per-engine queues:
  - dst rows use a uniform 8-slot layout (virtual-row splitting for
    deg>8 nodes) so softmax weights for ALL edges are computed by ~10
    flat vector instructions, with attention normalized up front.
  - per dst-block feature aggregation is one dma_gather + one multiply
    + one reduce; ELU is 3 flat instructions (max(z, exp(min(z,0))-1)).
  - layer-2 collapses to 15 linear functionals of x1 (vector-engine
    projection); its message passing reads a 15-wide table.
  - alpha_src/alpha_dst contractions are folded into the projection
    weights host-side, so tables are produced purely by matmuls.
"""

import sys

if "/opt/trn_rl_repo" not in sys.path:
    sys.path.insert(0, "/opt/trn_rl_repo")

import numpy as np

import concourse.bacc as bacc
import concourse.mybir as mybir
import concourse.tile as tile
from concourse.bass import IndirectOffsetOnAxis
from concourse.bass_utils import run_bass_kernel_spmd
from concourse.masks import make_identity

F32 = mybir.dt.float32
FP16 = mybir.dt.float16
I32 = mybir.dt.int32
I16 = mybir.dt.int16
AF = mybir.ActivationFunctionType
OP = mybir.AluOpType
AX = mybir.AxisListType

N_NODES, N_EDGES = 8192, 49152
IN_F, HID, H1, H2, OUT_F = 128, 64, 64, 5, 32
N_BONDS = 64
N_CORES = 8
NC_SHARD = N_NODES // N_CORES      # 1024 nodes per core
P = 128
NB = NC_SHARD // P                 # 8 main dst blocks per core
F1 = H1 * HID                      # 4096
T1W = F1 + 2 * H1                  # 4224: [h | alpha_src | alpha_dst]
KU = 8                             # uniform slots per dst row
NEG = -60000.0                     # additive logit mask (exp underflows to 0)
T2W = 16                           # t2 row: [hsum 5 | as2 5 | ad2 5 | pad]


def _wrap_idx(idx_flat: np.ndarray) -> np.ndarray:
    """[1024] -> [128, 64] in the swdge 'wrapped in 16 partitions,
    replicated across 8 gpsimd cores' index layout."""
    n = idx_flat.shape[0]
    w = idx_flat.reshape(n // 16, 16).T.astype(np.int16)
    return np.tile(w, (8, 1))


# ---------------------------------------------------------------- host side
def _prep(edge_index: np.ndarray):
    src = edge_index[0].astype(np.int64)
    dst = edge_index[1].astype(np.int64)
    deg = np.bincount(dst, minlength=N_NODES) + 1   # + self loop
    # neighbor lists: self first, then in-edge sources in edge order
    order = np.argsort(dst, kind="stable")
    ssrc, sdst = src[order], dst[order]
    starts = np.searchsorted(sdst, np.arange(N_NODES))

    # per-core degree-descending order
    sortpos = np.empty(N_NODES, np.int64)   # node -> sorted pos within shard
    bypos = np.empty(N_NODES, np.int64)     # global sorted pos -> node
    for c in range(N_CORES):
        lo = c * NC_SHARD
        o = np.argsort(-deg[lo:lo + NC_SHARD], kind="stable")
        sortpos[lo + o] = np.arange(NC_SHARD)
        bypos[lo + np.arange(NC_SHARD)] = lo + o

    # extra-row structure (consistent across cores): for the j-th extra row
    # (j>=1), dsts with deg > j*KU occupy sorted-pos prefix [0, n_j).
    exlist = []                              # (m_block, j) per extra block
    j = 1
    while True:
        n_j = 0
        for c in range(N_CORES):
            d = deg[bypos[c * NC_SHARD:(c + 1) * NC_SHARD]]
            n_j = max(n_j, int((d > j * KU).sum()))
        if n_j == 0:
            break
        nb_j = (n_j + P - 1) // P
        for m in range(nb_j):
            exlist.append((m, j))
        j += 1
    nblk = NB + len(exlist)

    # slot tables: srcidx[core][B, KU, p] (global node id), mask 0/-60000
    srcidx = np.zeros((N_CORES, nblk, KU, P), np.int64)
    maskf = np.full((N_CORES, nblk, KU), NEG, np.float32)[:, :, :, None] \
        .repeat(P, axis=3)
    for c in range(N_CORES):
        lo = c * NC_SHARD
        for i in range(NC_SHARD):
            node = bypos[lo + i]
            nbrs = np.concatenate(
                [[node], ssrc[starts[node]:starts[node] + deg[node] - 1]])
            p, b = i % P, i // P
            rows = [(b, 0)]
            for t, (m, jj) in enumerate(exlist):
                if m == b and deg[node] > jj * KU:
                    rows.append((NB + t, jj))
            for (B, jj) in rows:
                seg = nbrs[jj * KU:(jj + 1) * KU]
                srcidx[c, B, 0:len(seg), p] = seg
                maskf[c, B, 0:len(seg), p] = 0.0
    return exlist, nblk, srcidx, maskf, sortpos, bypos


def _make_core_inputs(inputs, prep, c, urep, w1ext):
    exlist, nblk, srcidx, maskf, sortpos, bypos = prep
    x = np.asarray(inputs["x"], np.float32)
    # L1 gather indices reference the table in original node order;
    # L2/finale reference sorted-global positions (s/t2 rows are sorted).
    g2pos = (np.arange(N_NODES) // NC_SHARD) * NC_SHARD + sortpos
    idx1 = np.concatenate([
        _wrap_idx(srcidx[c, B].reshape(KU * P)) for B in range(nblk)],
        axis=1)                                   # [128, nblk*64]
    idx2 = np.concatenate([
        _wrap_idx(g2pos[srcidx[c, B].reshape(KU * P)]) for B in range(nblk)],
        axis=1)
    lr = np.concatenate([np.asarray(inputs["lefts"], np.int64),
                         np.asarray(inputs["rights"], np.int64)])
    return {
        "xT": np.ascontiguousarray(
            x[c * NC_SHARD:(c + 1) * NC_SHARD].T).astype(np.float16),
        "W1ext": w1ext,
        "Urep": urep,
        "idx1": idx1,
        "idx2": idx2,
        "maskf": np.ascontiguousarray(
            maskf[c].transpose(2, 0, 1).reshape(P, nblk * KU))
            .astype(np.float32),
        "lridx": g2pos[lr].astype(np.int32).reshape(2 * N_BONDS, 1),
    }


def _fold_weights(inputs):
    W1 = np.asarray(inputs["W1"], np.float32)
    W2 = np.asarray(inputs["W2"], np.float32)
    as1 = np.asarray(inputs["att_src1"], np.float32)
    ad1 = np.asarray(inputs["att_dst1"], np.float32)
    as2 = np.asarray(inputs["att_src2"], np.float32)
    ad2 = np.asarray(inputs["att_dst2"], np.float32)
    W1r = W1.reshape(H1, HID, IN_F)
    A1s = np.einsum("hci,hc->ih", W1r, as1)        # [128, 64]
    A1d = np.einsum("hci,hc->ih", W1r, ad1)
    w1ext = np.concatenate([W1.T, A1s, A1d], axis=1).astype(np.float16)
    W2r = W2.reshape(H2, OUT_F, F1)
    Usum = W2r.sum(axis=1).T / H2                  # [4096, 5]
    A2s = np.einsum("hcj,hc->jh", W2r, as2)
    A2d = np.einsum("hcj,hc->jh", W2r, ad2)
    U = np.concatenate([Usum, A2s, A2d], axis=1)   # [4096, 15]
    urep = np.ascontiguousarray(
        np.broadcast_to(U.T.reshape(1, 15 * F1), (P, 15 * F1))
    ).astype(np.float16)
    return w1ext, urep


# ------------------------------------------------------------- device side
def _build_program(exlist, nblk, reps: int = 1, upto: int = 99) -> bacc.Bacc:
    nc = bacc.Bacc("TRN2", target_bir_lowering=False, debug=False,
                   num_devices=N_CORES)
    NW = nblk * KU                          # total slots per partition

    xT = nc.dram_tensor("xT", [P, NC_SHARD], FP16, kind="ExternalInput")
    W1ext = nc.dram_tensor("W1ext", [P, T1W], FP16, kind="ExternalInput")
    Urep = nc.dram_tensor("Urep", [P, 15 * F1], FP16, kind="ExternalInput")
    idx1 = nc.dram_tensor("idx1", [P, nblk * 64], I16, kind="ExternalInput")
    idx2 = nc.dram_tensor("idx2", [P, nblk * 64], I16, kind="ExternalInput")
    maskf = nc.dram_tensor("maskf", [P, NW], F32, kind="ExternalInput")
    lridx = nc.dram_tensor("lridx", [2 * N_BONDS, 1], I32,
                           kind="ExternalInput")
    y = nc.dram_tensor("y", [N_BONDS], F32, kind="ExternalOutput")

    rg = [list(range(N_CORES))]
    # extra-block merge ranges: contiguous runs of extra blocks whose main
    # blocks are also contiguous ascending
    merges = []
    t = 0
    while t < len(exlist):
        t2_ = t
        while (t2_ + 1 < len(exlist)
               and exlist[t2_ + 1][0] == exlist[t2_][0] + 1
               and exlist[t2_ + 1][1] == exlist[t2_][1]):
            t2_ += 1
        merges.append((NB + t, NB + t2_ + 1, exlist[t][0]))
        t = t2_ + 1

    with tile.TileContext(nc, num_cores=N_CORES) as tc:
        with (
            tc.tile_pool(name="consts", bufs=1) as cpool,
            tc.tile_pool(name="small", bufs=1) as mpool,
            tc.tile_pool(name="dram", bufs=1, space="DRAM") as dpool,
        ):
            idx1_s = cpool.tile([P, nblk * 64], I16)
            idx2_s = cpool.tile([P, nblk * 64], I16)
            mask_s = cpool.tile([P, NW], F32)
            lr_s = cpool.tile([2 * N_BONDS, 1], I32)
            ident_s = cpool.tile([P, P], F32)
            xT_s = cpool.tile([P, NC_SHARD], FP16)
            w1e_s = cpool.tile([P, T1W], FP16)
            for dt_, st_ in [(idx1_s, idx1), (idx2_s, idx2),
                             (mask_s, maskf), (lr_s, lridx),
                             (xT_s, xT), (w1e_s, W1ext)]:
                nc.sync.dma_start(dt_[:], st_[:])
            make_identity(nc, ident_s[:])

            for _rep in range(reps):
                t1_loc = dpool.tile([NC_SHARD, T1W], FP16, tag="t1l")
                t1_full = dpool.tile([N_NODES, T1W], FP16,
                                     addr_space="Shared", tag="t1f")
                t2_loc = dpool.tile([NC_SHARD, P], FP16, tag="t2l")
                t2_full = dpool.tile([N_NODES, P], FP16,
                                     addr_space="Shared", tag="t2f")
                s_loc = dpool.tile([NC_SHARD, 1], F32, tag="sl")
                s_full = dpool.tile([N_NODES, 1], F32,
                                    addr_space="Shared", tag="sf")

                # ---------------- phase A: t1 rows = x @ [W1.T | A1s | A1d]
                with tc.tile_pool(name="pha", bufs=1) as hpool:
                    h_all = hpool.tile([P, NB, T1W], FP16, tag="hall")
                    # batched alpha projections: 8 blocks into 2 psum banks
                    with tc.tile_pool(name="psa", bufs=1,
                                      space="PSUM") as papool:
                        for half in range(2):
                            ps_a = papool.tile([P, 512], F32,
                                               tag=f"psa{half}")
                            for q in range(4):
                                b = half * 4 + q
                                nc.tensor.matmul(
                                    ps_a[:, q * 128:(q + 1) * 128],
                                    lhsT=xT_s[:, b * P:(b + 1) * P],
                                    rhs=w1e_s[:, F1:T1W],
                                    start=True, stop=True)
                            nc.scalar.activation(
                                h_all[:, half * 4:(half + 1) * 4, F1:T1W],
                                ps_a[:].rearrange("p (b w) -> p b w", w=128),
                                AF.Copy)
                    # main projections: 8 psum banks per block
                    with tc.tile_pool(name="psh", bufs=1,
                                      space="PSUM") as phpool:
                        for b in range(NB):
                            ps_h = phpool.tile([P, F1], F32, tag="psh")
                            for j in range(8):
                                nc.tensor.matmul(
                                    ps_h[:, j * 512:(j + 1) * 512],
                                    lhsT=xT_s[:, b * P:(b + 1) * P],
                                    rhs=w1e_s[:, j * 512:(j + 1) * 512],
                                    start=True, stop=True)
                            nc.scalar.activation(h_all[:, b, 0:F1], ps_h[:],
                                                 AF.Copy)
                    nc.sync.dma_start(
                        t1_loc[:].rearrange("(b p) w -> p b w", p=P),
                        h_all[:])

                nc.gpsimd.collective_compute(
                    "AllGather", OP.bypass, ins=[t1_loc.opt()],
                    outs=[t1_full.opt()], replica_groups=rg)

                if upto < 2:
                    continue
                # ---------------- L1 softmax weights (flat, all slots)
                with (
                    tc.tile_pool(name="l1x", bufs=1) as xpool,
                    tc.tile_pool(name="l1ad", bufs=1) as apool,
                ):
                    x1_all = xpool.tile([P, NB, F1], FP16, tag="x1")
                    alpha = apool.tile([P, nblk, KU, H1], FP16, tag="alp")
                    den = apool.tile([P, nblk, H1], F32, tag="den")
                    with tc.tile_pool(name="l1w", bufs=1) as wpool:
                        asad = wpool.tile([P, nblk, KU, 2 * H1], FP16,
                                          tag="asad")
                        for B in range(nblk):
                            nc.gpsimd.dma_gather(
                                out_ap=asad[:, B],
                                in_ap=t1_full[:, F1:T1W],
                                idxs_ap=idx1_s[:, B * 64:(B + 1) * 64],
                                num_idxs=KU * P, num_idxs_reg=KU * P,
                                elem_size=2 * H1, elem_step=T1W)
                        wv = wpool.tile([P, nblk, KU, H1], F32, tag="wv")
                        # logits = as[src] + ad[dst]; dst ad = self slot 0
                        nc.vector.tensor_tensor(
                            out=wv[:, 0:NB], in0=asad[:, 0:NB, :, 0:H1],
                            in1=asad[:, 0:NB, 0:1, H1:2 * H1]
                                .broadcast_to([P, NB, KU, H1]), op=OP.add)
                        for t, (m, _j) in enumerate(exlist):
                            B = NB + t
                            nc.vector.tensor_tensor(
                                out=wv[:, B:B + 1],
                                in0=asad[:, B:B + 1, :, 0:H1],
                                in1=asad[:, m:m + 1, 0:1, H1:2 * H1]
                                    .broadcast_to([P, 1, KU, H1]),
                                op=OP.add)
                        nc.vector.tensor_tensor(
                            out=wv, in0=wv,
                            in1=mask_s[:].rearrange("p (b k) -> p b k", k=KU)
                                .unsqueeze(3)
                                .broadcast_to([P, nblk, KU, H1]),
                            op=OP.add)
                        nc.vector.scalar_tensor_tensor(
                            out=wv[:], in0=wv[:], scalar=0.2,
                            in1=wv[:], op0=OP.mult, op1=OP.max)
                        nc.scalar.activation(alpha[:], wv[:], AF.Exp)
                        nc.vector.tensor_reduce(
                            out=den[:], in_=alpha[:].transpose([0, 1, 3, 2]),
                            axis=AX.X, op=OP.add)
                        for (e0, e1, m0) in merges:
                            nc.vector.tensor_tensor(
                                out=den[:, m0:m0 + (e1 - e0)],
                                in0=den[:, m0:m0 + (e1 - e0)],
                                in1=den[:, e0:e1], op=OP.add)
                        nc.vector.reciprocal(den[:], den[:])
                        nc.vector.tensor_tensor(
                            out=alpha[:, 0:NB], in0=alpha[:, 0:NB],
                            in1=den[:, 0:NB].unsqueeze(2)
                                .broadcast_to([P, NB, KU, H1]),
                            op=OP.mult)
                        for t, (m, _j) in enumerate(exlist):
                            B = NB + t
                            nc.vector.tensor_tensor(
                                out=alpha[:, B:B + 1], in0=alpha[:, B:B + 1],
                                in1=den[:, m:m + 1].unsqueeze(2)
                                    .broadcast_to([P, 1, KU, H1]),
                                op=OP.mult)

                    # -------- L1 feature aggregation + per-block ELU
                    # (main-block order with extras processed right after
                    # their main block so `num` can accumulate)
                    border = []
                    for m in range(NB):
                        border.append((m, m))
                        for t, (mm, _j) in enumerate(exlist):
                            if mm == m:
                                border.append((NB + t, m))
                    with nc.allow_low_precision(reason="fp16 8-term sums"):
                        with tc.tile_pool(name="l1f", bufs=1) as fpool:
                            num = fpool.tile([P, F1], F32, tag="num")
                            pnum = fpool.tile([P, F1], F32, tag="pn")
                            for (B, m) in border:
                                g = fpool.tile([P, KU, F1], FP16, tag="g")
                                nc.gpsimd.dma_gather(
                                    out_ap=g[:],
                                    in_ap=t1_full[:, 0:F1],
                                    idxs_ap=idx1_s[:, B * 64:(B + 1) * 64],
                                    num_idxs=KU * P, num_idxs_reg=KU * P,
                                    elem_size=F1, elem_step=T1W)
                                nc.vector.tensor_tensor(
                                    out=g[:].rearrange(
                                        "p k (h c) -> p k h c", c=HID),
                                    in0=g[:].rearrange(
                                        "p k (h c) -> p k h c", c=HID),
                                    in1=alpha[:, B].unsqueeze(3)
                                        .broadcast_to([P, KU, H1, HID]),
                                    op=OP.mult)
                                first = B == m
                                nc.vector.tensor_reduce(
                                    out=num[:] if first else pnum[:],
                                    in_=g[:].transpose([0, 2, 1]),
                                    axis=AX.X, op=OP.add)
                                if not first:
                                    nc.vector.tensor_tensor(
                                        out=num[:], in0=num[:],
                                        in1=pnum[:], op=OP.add)
                                last = (B, m) == border[-1] or \
                                    border[border.index((B, m)) + 1][1] != m
                                if last:
                                    # ELU: x1 = max(z, exp(min(z,0)) - 1)
                                    nc.scalar.activation(
                                        x1_all[:, m], num[:],
                                        AF.Relu, scale=-1.0)
                                    nc.scalar.activation(
                                        x1_all[:, m], x1_all[:, m],
                                        AF.Exp, scale=-1.0)
                                    nc.vector.scalar_tensor_tensor(
                                        out=x1_all[:, m],
                                        in0=x1_all[:, m], scalar=-1.0,
                                        in1=num[:], op0=OP.add, op1=OP.max)

                    # ------------ layer-2 projection (vector): t2 = x1 @ U
                    with tc.tile_pool(name="prj", bufs=1) as jpool:
                        t2tmp = mpool.tile([P, NB, T2W], F32, tag="t2t")
                        for q in range(4):
                            urep_s = jpool.tile([P, 4 * F1], FP16, tag="ur")
                            nj = 4 if q < 3 else 3
                            nc.sync.dma_start(
                                urep_s[:, 0:nj * F1],
                                Urep[:, 4 * q * F1:(4 * q + nj) * F1])
                            for jj in range(nj):
                                j = 4 * q + jj
                                prod = jpool.tile([P, NB, F1], FP16,
                                                  tag="pr")
                                nc.vector.tensor_tensor(
                                    out=prod[:], in0=x1_all[:],
                                    in1=urep_s[:, jj * F1:(jj + 1) * F1]
                                        .unsqueeze(1)
                                        .broadcast_to([P, NB, F1]),
                                    op=OP.mult)
                                nc.vector.tensor_reduce(
                                    out=t2tmp[:, :, j], in_=prod[:],
                                    axis=AX.X, op=OP.add)
                        t2sb = mpool.tile([P, NB, T2W], FP16, tag="t2s")
                        nc.vector.tensor_copy(t2sb[:], t2tmp[:])
                        nc.sync.dma_start(
                            t2_loc[:, 0:T2W]
                                .rearrange("(b p) w -> p b w", p=P),
                            t2sb[:])

                if upto < 4:
                    continue
                nc.gpsimd.collective_compute(
                    "AllGather", OP.bypass, ins=[t2_loc.opt()],
                    outs=[t2_full.opt()], replica_groups=rg)

                # ---------------- layer-2 message passing (flat)
                with tc.tile_pool(name="l2", bufs=1) as lpool:
                    g2 = lpool.tile([P, nblk, KU, P], FP16, tag="g2")
                    for B in range(nblk):
                        nc.gpsimd.dma_gather(
                            out_ap=g2[:, B],
                            in_ap=t2_full[:],
                            idxs_ap=idx2_s[:, B * 64:(B + 1) * 64],
                            num_idxs=KU * P, num_idxs_reg=KU * P,
                            elem_size=P)
                    w2 = lpool.tile([P, nblk, KU, H2], F32, tag="w2")
                    nc.vector.tensor_tensor(
                        out=w2[:, 0:NB], in0=g2[:, 0:NB, :, H2:2 * H2],
                        in1=g2[:, 0:NB, 0:1, 2 * H2:3 * H2]
                            .broadcast_to([P, NB, KU, H2]), op=OP.add)
                    for t, (m, _j) in enumerate(exlist):
                        B = NB + t
                        nc.vector.tensor_tensor(
                            out=w2[:, B:B + 1],
                            in0=g2[:, B:B + 1, :, H2:2 * H2],
                            in1=g2[:, m:m + 1, 0:1, 2 * H2:3 * H2]
                                .broadcast_to([P, 1, KU, H2]), op=OP.add)
                    nc.vector.tensor_tensor(
                        out=w2, in0=w2,
                        in1=mask_s[:].rearrange("p (b k) -> p b k", k=KU)
                            .unsqueeze(3).broadcast_to([P, nblk, KU, H2]),
                        op=OP.add)
                    nc.vector.scalar_tensor_tensor(
                        out=w2[:], in0=w2[:], scalar=0.2,
                        in1=w2[:], op0=OP.mult, op1=OP.max)
                    w2h = lpool.tile([P, nblk, KU, H2], FP16, tag="w2h")
                    nc.scalar.activation(w2h[:], w2[:], AF.Exp)
                    den2 = mpool.tile([P, nblk, H2], F32, tag="dn2")
                    nc.vector.tensor_reduce(
                        out=den2[:], in_=w2h[:].transpose([0, 1, 3, 2]),
                        axis=AX.X, op=OP.add)
                    nc.vector.tensor_tensor(
                        out=g2[:, :, :, 0:H2], in0=g2[:, :, :, 0:H2],
                        in1=w2h[:], op=OP.mult)
                    num2 = mpool.tile([P, nblk, H2], F32, tag="nm2")
                    nc.vector.tensor_reduce(
                        out=num2[:],
                        in_=g2[:, :, :, 0:H2].transpose([0, 1, 3, 2]),
                        axis=AX.X, op=OP.add)
                    for (e0, e1, m0) in merges:
                        w_ = e1 - e0
                        nc.vector.tensor_tensor(
                            out=den2[:, m0:m0 + w_], in0=den2[:, m0:m0 + w_],
                            in1=den2[:, e0:e1], op=OP.add)
                        nc.vector.tensor_tensor(
                            out=num2[:, m0:m0 + w_], in0=num2[:, m0:m0 + w_],
                            in1=num2[:, e0:e1], op=OP.add)
                    x2 = mpool.tile([P, NB, H2], F32, tag="x2")
                    nc.vector.reciprocal(den2[:, 0:NB], den2[:, 0:NB])
                    nc.vector.tensor_tensor(
                        out=x2[:], in0=num2[:, 0:NB], in1=den2[:, 0:NB],
                        op=OP.mult)
                    sa = mpool.tile([P, NB], F32, tag="sa")
                    nc.vector.tensor_reduce(
                        out=sa[:], in_=x2[:], axis=AX.X, op=OP.add)
                    nc.sync.dma_start(
                        s_loc[:].rearrange("(b p) o -> p (b o)", p=P),
                        sa[:])

                nc.gpsimd.collective_compute(
                    "AllGather", OP.bypass, ins=[s_loc.opt()],
                    outs=[s_full.opt()], replica_groups=rg)

                if upto < 6:
                    continue
                # ---------------- finale: bond scores + softmax
                gl = mpool.tile([2 * N_BONDS, 1], F32, tag="gl")
                nc.gpsimd.indirect_dma_start(
                    out=gl[:], out_offset=None, in_=s_full[:],
                    in_offset=IndirectOffsetOnAxis(ap=lr_s[:, 0:1], axis=0))
                sc = mpool.tile([1, N_BONDS], F32, tag="sc")
                scb = mpool.tile([1, 2 * N_BONDS], F32, tag="scb")
                with tc.tile_pool(name="psf", bufs=1,
                                  space="PSUM") as pfpool:
                    ps_t = pfpool.tile([P, 512], F32, tag="pst")
                    nc.tensor.transpose(
                        out=ps_t[0:1, 0:P], in_=gl[:],
                        identity=ident_s[:])
                    nc.scalar.activation(scb[:], ps_t[0:1, 0:2 * N_BONDS],
                                         AF.Copy)
                nc.vector.tensor_tensor(
                    out=sc[:], in0=scb[:, 0:N_BONDS],
                    in1=scb[:, N_BONDS:2 * N_BONDS], op=OP.add)
                es = mpool.tile([1, N_BONDS], F32, tag="es")
                nc.scalar.activation(es[:], sc[:], AF.Exp)
                ssum = mpool.tile([1, 1], F32, tag="ss")
                nc.vector.tensor_reduce(
                    out=ssum[:], in_=es[:], axis=AX.X, op=OP.add)
                ys = mpool.tile([1, N_BONDS], F32, tag="ys")
                nc.vector.reciprocal(ssum[:], ssum[:])
                nc.vector.tensor_tensor(
                    out=ys[:], in0=es[:],
                    in1=ssum[:].to_broadcast([1, N_BONDS]), op=OP.mult)
                nc.sync.dma_start(y.ap().unsqueeze(0), ys[:])

    nc.compile()
    return nc


_PROGRAM_CACHE: dict = {}


def kernel(**inputs) -> np.ndarray:
    prep = _prep(np.asarray(inputs["edge_index"], np.int64))
    exlist, nblk = prep[0], prep[1]
    key = (tuple(exlist), nblk)
    if key not in _PROGRAM_CACHE:
        _PROGRAM_CACHE[key] = _build_program(exlist, nblk)
    nc = _PROGRAM_CACHE[key]
    w1ext, urep = _fold_weights(inputs)
    in_maps = [_make_core_inputs(inputs, prep, c, urep, w1ext)
               for c in range(N_CORES)]
    res = run_bass_kernel_spmd(nc, in_maps, core_ids=list(range(N_CORES)))
    return res.results[0]["y"]


if __name__ == "__main__":
    import jax

    import reference

    with jax.default_device(jax.devices("cpu")[0]):
        inputs = {k: np.asarray(v) for k, v in reference.setup_inputs().items()}
        expected = np.asarray(reference.reference(**reference.setup_inputs()))
    actual = kernel(**inputs)
    rel = np.abs(actual - expected).max() / np.abs(expected).max()
    print("Relative error:", rel)


# revision 13
# speedup vs baseline: 1.4797x; 1.4797x over previous
"""Two-layer GAT (gnn_message_passing) on 8 Trainium2 NeuronCores.

Sharding: nodes are split into 8 contiguous shards of 1024 (one per core);
each core owns every edge whose destination lies in its shard.  The halo
exchange is an on-device AllGather of the node feature table
[h | alpha_src | alpha_dst]; cores resolve incident edges with
InstDMAGatherAnt (swdge dma_gather: up to 1024 arbitrary table rows per
instruction, ~125us nearly independent of payload width).

Environment cost model (measured): ~50-80us fixed per instruction on every
engine queue; dma_gather ~125-145us for <=1024 rows of any width;
AllGather ~110us + bytes/4.5GB/s.  The kernel therefore minimizes
instruction count and balances the per-engine queues:
  - dst rows use a uniform 8-slot layout (virtual-row splitting for
    deg>8 nodes) so softmax weights for ALL edges are computed by ~10
    flat vector instructions, with attention normalized up front.
  - per dst-block feature aggregation is one dma_gather + one multiply
    + one reduce; ELU is 3 flat instructions (max(z, exp(min(z,0))-1)).
  - layer-2 collapses to 15 linear functionals of x1 (vector-engine
    projection); its message passing reads a 15-wide table.
  - alpha_src/alpha_dst contractions are folded into the projection
    weights host-side, so tables are produced purely by matmuls.
"""

import sys

if "/opt/trn_rl_repo" not in sys.path:
    sys.path.insert(0, "/opt/trn_rl_repo")

import numpy as np

import concourse.bacc as bacc
import concourse.mybir as mybir
import concourse.tile as tile
from concourse.bass import IndirectOffsetOnAxis
from concourse.bass_utils import run_bass_kernel_spmd
from concourse.masks import make_identity

F32 = mybir.dt.float32
FP16 = mybir.dt.float16
I32 = mybir.dt.int32
I16 = mybir.dt.int16
AF = mybir.ActivationFunctionType
OP = mybir.AluOpType
AX = mybir.AxisListType

N_NODES, N_EDGES = 8192, 49152
IN_F, HID, H1, H2, OUT_F = 128, 64, 64, 5, 32
N_BONDS = 64
N_CORES = 8
NC_SHARD = N_NODES // N_CORES      # 1024 nodes per core
P = 128
NB = NC_SHARD // P                 # 8 main dst blocks per core
F1 = H1 * HID                      # 4096
T1W = F1 + 2 * H1                  # 4224: [h | alpha_src | alpha_dst]
KU = 8                             # uniform slots per dst row
NEG = -60000.0                     # additive logit mask (exp underflows to 0)
T2W = 16                           # t2 row: [hsum 5 | as2 5 | ad2 5 | pad]


def _wrap_idx(idx_flat: np.ndarray) -> np.ndarray:
    """[1024] -> [128, 64] in the swdge 'wrapped in 16 partitions,
    replicated across 8 gpsimd cores' index layout."""
    n = idx_flat.shape[0]
    w = idx_flat.reshape(n // 16, 16).T.astype(np.int16)
    return np.tile(w, (8, 1))


# ---------------------------------------------------------------- host side
def _prep(edge_index: np.ndarray):
    src = edge_index[0].astype(np.int64)
    dst = edge_index[1].astype(np.int64)
    deg = np.bincount(dst, minlength=N_NODES) + 1   # + self loop
    # neighbor lists: self first, then in-edge sources in edge order
    order = np.argsort(dst, kind="stable")
    ssrc, sdst = src[order], dst[order]
    starts = np.searchsorted(sdst, np.arange(N_NODES))

    # per-core degree-descending order
    sortpos = np.empty(N_NODES, np.int64)   # node -> sorted pos within shard
    bypos = np.empty(N_NODES, np.int64)     # global sorted pos -> node
    for c in range(N_CORES):
        lo = c * NC_SHARD
        o = np.argsort(-deg[lo:lo + NC_SHARD], kind="stable")
        sortpos[lo + o] = np.arange(NC_SHARD)
        bypos[lo + np.arange(NC_SHARD)] = lo + o

    # extra-row structure (consistent across cores): for the j-th extra row
    # (j>=1), dsts with deg > j*KU occupy sorted-pos prefix [0, n_j).
    exlist = []                              # (m_block, j) per extra block
    j = 1
    while True:
        n_j = 0
        for c in range(N_CORES):
            d = deg[bypos[c * NC_SHARD:(c + 1) * NC_SHARD]]
            n_j = max(n_j, int((d > j * KU).sum()))
        if n_j == 0:
            break
        nb_j = (n_j + P - 1) // P
        for m in range(nb_j):
            exlist.append((m, j))
        j += 1
    nblk = NB + len(exlist)

    # slot tables: srcidx[core][B, KU, p] (global node id), mask 0/-60000
    srcidx = np.zeros((N_CORES, nblk, KU, P), np.int64)
    maskf = np.full((N_CORES, nblk, KU), NEG, np.float32)[:, :, :, None] \
        .repeat(P, axis=3)
    for c in range(N_CORES):
        lo = c * NC_SHARD
        for i in range(NC_SHARD):
            node = bypos[lo + i]
            nbrs = np.concatenate(
                [[node], ssrc[starts[node]:starts[node] + deg[node] - 1]])
            p, b = i % P, i // P
            rows = [(b, 0)]
            for t, (m, jj) in enumerate(exlist):
                if m == b and deg[node] > jj * KU:
                    rows.append((NB + t, jj))
            for (B, jj) in rows:
                seg = nbrs[jj * KU:(jj + 1) * KU]
                srcidx[c, B, 0:len(seg), p] = seg
                maskf[c, B, 0:len(seg), p] = 0.0
    return exlist, nblk, srcidx, maskf, sortpos, bypos


def _make_core_inputs(inputs, prep, c, urep, w1ext):
    exlist, nblk, srcidx, maskf, sortpos, bypos = prep
    x = np.asarray(inputs["x"], np.float32)
    # L1 gather indices reference the table in original node order;
    # L2/finale reference sorted-global positions (s/t2 rows are sorted).
    g2pos = (np.arange(N_NODES) // NC_SHARD) * NC_SHARD + sortpos
    idx1 = np.concatenate([
        _wrap_idx(srcidx[c, B].reshape(KU * P)) for B in range(nblk)],
        axis=1)                                   # [128, nblk*64]
    idx2 = np.concatenate([
        _wrap_idx(g2pos[srcidx[c, B].reshape(KU * P)]) for B in range(nblk)],
        axis=1)
    lr = np.concatenate([np.asarray(inputs["lefts"], np.int64),
                         np.asarray(inputs["rights"], np.int64)])
    return {
        "xT": np.ascontiguousarray(
            x[c * NC_SHARD:(c + 1) * NC_SHARD].T).astype(np.float16),
        "W1ext": w1ext,
        "Urep": urep,
        "idx1": idx1,
        "idx2": idx2,
        "maskf": np.ascontiguousarray(
            maskf[c].transpose(2, 0, 1).reshape(P, nblk * KU))
            .astype(np.float32),
        "lridx": g2pos[lr].astype(np.int32).reshape(2 * N_BONDS, 1),
    }


def _fold_weights(inputs):
    W1 = np.asarray(inputs["W1"], np.float32)
    W2 = np.asarray(inputs["W2"], np.float32)
    as1 = np.asarray(inputs["att_src1"], np.float32)
    ad1 = np.asarray(inputs["att_dst1"], np.float32)
    as2 = np.asarray(inputs["att_src2"], np.float32)
    ad2 = np.asarray(inputs["att_dst2"], np.float32)
    W1r = W1.reshape(H1, HID, IN_F)
    A1s = np.einsum("hci,hc->ih", W1r, as1)        # [128, 64]
    A1d = np.einsum("hci,hc->ih", W1r, ad1)
    w1ext = np.concatenate([W1.T, A1s, A1d], axis=1).astype(np.float16)
    W2r = W2.reshape(H2, OUT_F, F1)
    Usum = W2r.sum(axis=1).T / H2                  # [4096, 5]
    A2s = np.einsum("hcj,hc->jh", W2r, as2)
    A2d = np.einsum("hcj,hc->jh", W2r, ad2)
    U = np.concatenate([Usum, A2s, A2d], axis=1)   # [4096, 15]
    urep = np.ascontiguousarray(
        np.broadcast_to(U.T.reshape(1, 15 * F1), (P, 15 * F1))
    ).astype(np.float16)
    return w1ext, urep


# ------------------------------------------------------------- device side
def _build_program(exlist, nblk, reps: int = 1, upto: int = 99) -> bacc.Bacc:
    nc = bacc.Bacc("TRN2", target_bir_lowering=False, debug=False,
                   num_devices=N_CORES)
    NW = nblk * KU                          # total slots per partition

    xT = nc.dram_tensor("xT", [P, NC_SHARD], FP16, kind="ExternalInput")
    W1ext = nc.dram_tensor("W1ext", [P, T1W], FP16, kind="ExternalInput")
    Urep = nc.dram_tensor("Urep", [P, 15 * F1], FP16, kind="ExternalInput")
    idx1 = nc.dram_tensor("idx1", [P, nblk * 64], I16, kind="ExternalInput")
    idx2 = nc.dram_tensor("idx2", [P, nblk * 64], I16, kind="ExternalInput")
    maskf = nc.dram_tensor("maskf", [P, NW], F32, kind="ExternalInput")
    lridx = nc.dram_tensor("lridx", [2 * N_BONDS, 1], I32,
                           kind="ExternalInput")
    y = nc.dram_tensor("y", [N_BONDS], F32, kind="ExternalOutput")

    rg = [list(range(N_CORES))]
    # extra-block merge ranges: contiguous runs of extra blocks whose main
    # blocks are also contiguous ascending
    merges = []
    t = 0
    while t < len(exlist):
        t2_ = t
        while (t2_ + 1 < len(exlist)
               and exlist[t2_ + 1][0] == exlist[t2_][0] + 1
               and exlist[t2_ + 1][1] == exlist[t2_][1]):
            t2_ += 1
        merges.append((NB + t, NB + t2_ + 1, exlist[t][0]))
        t = t2_ + 1

    with tile.TileContext(nc, num_cores=N_CORES) as tc:
        with (
            tc.tile_pool(name="consts", bufs=1) as cpool,
            tc.tile_pool(name="small", bufs=1) as mpool,
            tc.tile_pool(name="dram", bufs=1, space="DRAM") as dpool,
        ):
            idx1_s = cpool.tile([P, nblk * 64], I16)
            idx2_s = cpool.tile([P, nblk * 64], I16)
            mask_s = cpool.tile([P, NW], F32)
            lr_s = cpool.tile([2 * N_BONDS, 1], I32)
            ident_s = cpool.tile([P, P], F32)
            xT_s = cpool.tile([P, NC_SHARD], FP16)
            w1e_s = cpool.tile([P, T1W], FP16)
            for dt_, st_ in [(idx1_s, idx1), (idx2_s, idx2),
                             (mask_s, maskf), (lr_s, lridx),
                             (xT_s, xT), (w1e_s, W1ext)]:
                nc.sync.dma_start(dt_[:], st_[:])
            make_identity(nc, ident_s[:])

            for _rep in range(reps):
                ad_loc = dpool.tile([NC_SHARD, 2 * H1], FP16, tag="adl")
                ad_full = dpool.tile([N_NODES, 2 * H1], FP16,
                                     addr_space="Shared", tag="adf")
                h_loc = dpool.tile([NC_SHARD, F1], FP16, tag="hl")
                h_full = dpool.tile([N_NODES, F1], FP16,
                                    addr_space="Shared", tag="hf")
                t2_loc = dpool.tile([NC_SHARD, P], FP16, tag="t2l")
                t2_full = dpool.tile([N_NODES, P], FP16,
                                     addr_space="Shared", tag="t2f")
                s_loc = dpool.tile([NC_SHARD, 1], F32, tag="sl")
                s_full = dpool.tile([N_NODES, 1], F32,
                                    addr_space="Shared", tag="sf")

                # -------- phase A part 1: alpha projections, early AG
                with tc.tile_pool(name="l1ad", bufs=1) as apool:
                  alpha = apool.tile([P, nblk, KU, H1], FP16, tag="alp")
                  den = apool.tile([P, nblk, H1], F32, tag="den")
                  with tc.tile_pool(name="pha", bufs=1) as hpool:
                    h_all = hpool.tile([P, NB, F1], FP16, tag="hall")
                    asad_sb = hpool.tile([P, NB, 2 * H1], FP16, tag="asb")
                    with tc.tile_pool(name="psa", bufs=1,
                                      space="PSUM") as papool:
                        for half in range(2):
                            ps_a = papool.tile([P, 512], F32,
                                               tag=f"psa{half}")
                            for q in range(4):
                                b = half * 4 + q
                                nc.tensor.matmul(
                                    ps_a[:, q * 128:(q + 1) * 128],
                                    lhsT=xT_s[:, b * P:(b + 1) * P],
                                    rhs=w1e_s[:, F1:T1W],
                                    start=True, stop=True)
                            nc.scalar.activation(
                                asad_sb[:, half * 4:(half + 1) * 4, :],
                                ps_a[:].rearrange("p (b w) -> p b w", w=128),
                                AF.Copy)
                    nc.sync.dma_start(
                        ad_loc[:].rearrange("(b p) w -> p b w", p=P),
                        asad_sb[:])
                    nc.gpsimd.collective_compute(
                        "AllGather", OP.bypass, ins=[ad_loc.opt()],
                        outs=[ad_full.opt()], replica_groups=rg)

                    # ----- L1 softmax weights: overlap the h matmuls below
                    with tc.tile_pool(name="l1w", bufs=1) as wpool:
                        asad = wpool.tile([P, nblk, KU, 2 * H1], FP16,
                                          tag="asad")
                        for B in range(nblk):
                            nc.gpsimd.dma_gather(
                                out_ap=asad[:, B],
                                in_ap=ad_full[:],
                                idxs_ap=idx1_s[:, B * 64:(B + 1) * 64],
                                num_idxs=KU * P, num_idxs_reg=KU * P,
                                elem_size=2 * H1)
                        wv = wpool.tile([P, nblk, KU, H1], F32, tag="wv")
                        # logits = as[src] + ad[dst]; dst ad = self slot 0
                        nc.vector.tensor_tensor(
                            out=wv[:, 0:NB], in0=asad[:, 0:NB, :, 0:H1],
                            in1=asad[:, 0:NB, 0:1, H1:2 * H1]
                                .broadcast_to([P, NB, KU, H1]), op=OP.add)
                        for t, (m, _j) in enumerate(exlist):
                            B = NB + t
                            nc.vector.tensor_tensor(
                                out=wv[:, B:B + 1],
                                in0=asad[:, B:B + 1, :, 0:H1],
                                in1=asad[:, m:m + 1, 0:1, H1:2 * H1]
                                    .broadcast_to([P, 1, KU, H1]),
                                op=OP.add)
                        nc.vector.tensor_tensor(
                            out=wv, in0=wv,
                            in1=mask_s[:].rearrange("p (b k) -> p b k", k=KU)
                                .unsqueeze(3)
                                .broadcast_to([P, nblk, KU, H1]),
                            op=OP.add)
                        nc.vector.scalar_tensor_tensor(
                            out=wv[:], in0=wv[:], scalar=0.2,
                            in1=wv[:], op0=OP.mult, op1=OP.max)
                        nc.scalar.activation(alpha[:], wv[:], AF.Exp)
                        nc.vector.tensor_reduce(
                            out=den[:], in_=alpha[:].transpose([0, 1, 3, 2]),
                            axis=AX.X, op=OP.add)
                        for (e0, e1, m0) in merges:
                            nc.vector.tensor_tensor(
                                out=den[:, m0:m0 + (e1 - e0)],
                                in0=den[:, m0:m0 + (e1 - e0)],
                                in1=den[:, e0:e1], op=OP.add)
                        nc.vector.reciprocal(den[:], den[:])
                        nc.vector.tensor_tensor(
                            out=alpha[:, 0:NB], in0=alpha[:, 0:NB],
                            in1=den[:, 0:NB].unsqueeze(2)
                                .broadcast_to([P, NB, KU, H1]),
                            op=OP.mult)
                        for t, (m, _j) in enumerate(exlist):
                            B = NB + t
                            nc.vector.tensor_tensor(
                                out=alpha[:, B:B + 1], in0=alpha[:, B:B + 1],
                                in1=den[:, m:m + 1].unsqueeze(2)
                                    .broadcast_to([P, 1, KU, H1]),
                                op=OP.mult)

                    # -------- phase A part 2: h projections (tensor queue)
                    with tc.tile_pool(name="psh", bufs=1,
                                      space="PSUM") as phpool:
                        for b in range(NB):
                            for half in range(2):
                                ps_h = phpool.tile([P, F1 // 2], F32,
                                                   tag="psh", bufs=2)
                                for j in range(4):
                                    jj = half * 4 + j
                                    nc.tensor.matmul(
                                        ps_h[:, j * 512:(j + 1) * 512],
                                        lhsT=xT_s[:, b * P:(b + 1) * P],
                                        rhs=w1e_s[:, jj * 512:
                                                  (jj + 1) * 512],
                                        start=True, stop=True)
                                nc.scalar.activation(
                                    h_all[:, b,
                                          half * (F1 // 2):
                                          (half + 1) * (F1 // 2)],
                                    ps_h[:], AF.Copy)
                            if b == NB // 2 - 1:
                                nc.sync.dma_start(
                                    h_loc[0:NC_SHARD // 2]
                                    .rearrange("(b p) w -> p b w", p=P),
                                    h_all[:, 0:NB // 2])
                        nc.sync.dma_start(
                            h_loc[NC_SHARD // 2:NC_SHARD]
                            .rearrange("(b p) w -> p b w", p=P),
                            h_all[:, NB // 2:NB])
                  nc.gpsimd.collective_compute(
                      "AllGather", OP.bypass, ins=[h_loc.opt()],
                      outs=[h_full.opt()], replica_groups=rg)

                  if upto < 2:
                    continue
                  with tc.tile_pool(name="l1x", bufs=1) as xpool:
                    x1_all = xpool.tile([P, NB, F1], FP16, tag="x1")
                    # -------- L1 feature aggregation + per-block ELU
                    # (main-block order with extras processed right after
                    # their main block so `num` can accumulate)
                    border = []
                    for m in range(NB):
                        border.append((m, m))
                        for t, (mm, _j) in enumerate(exlist):
                            if mm == m:
                                border.append((NB + t, m))
                    with nc.allow_low_precision(reason="fp16 8-term sums"):
                        with tc.tile_pool(name="l1f", bufs=1) as fpool:
                            num = fpool.tile([P, F1], F32, tag="num")
                            pnum = fpool.tile([P, F1], F32, tag="pn")
                            for (B, m) in border:
                                g = fpool.tile([P, KU, F1], FP16, tag="g")
                                nc.gpsimd.dma_gather(
                                    out_ap=g[:],
                                    in_ap=h_full[:],
                                    idxs_ap=idx1_s[:, B * 64:(B + 1) * 64],
                                    num_idxs=KU * P, num_idxs_reg=KU * P,
                                    elem_size=F1)
                                nc.vector.tensor_tensor(
                                    out=g[:].rearrange(
                                        "p k (h c) -> p k h c", c=HID),
                                    in0=g[:].rearrange(
                                        "p k (h c) -> p k h c", c=HID),
                                    in1=alpha[:, B].unsqueeze(3)
                                        .broadcast_to([P, KU, H1, HID]),
                                    op=OP.mult)
                                first = B == m
                                nc.vector.tensor_reduce(
                                    out=num[:] if first else pnum[:],
                                    in_=g[:].transpose([0, 2, 1]),
                                    axis=AX.X, op=OP.add)
                                if not first:
                                    nc.vector.tensor_tensor(
                                        out=num[:], in0=num[:],
                                        in1=pnum[:], op=OP.add)
                                last = (B, m) == border[-1] or \
                                    border[border.index((B, m)) + 1][1] != m
                                if last:
                                    # ELU: x1 = max(z, exp(min(z,0)) - 1)
                                    nc.scalar.activation(
                                        x1_all[:, m], num[:],
                                        AF.Relu, scale=-1.0)
                                    nc.scalar.activation(
                                        x1_all[:, m], x1_all[:, m],
                                        AF.Exp, scale=-1.0)
                                    nc.vector.scalar_tensor_tensor(
                                        out=x1_all[:, m],
                                        in0=x1_all[:, m], scalar=-1.0,
                                        in1=num[:], op0=OP.add, op1=OP.max)

                    # ------------ layer-2 projection (vector): t2 = x1 @ U
                    with tc.tile_pool(name="prj", bufs=1) as jpool:
                        t2tmp = mpool.tile([P, NB, T2W], F32, tag="t2t")
                        for q in range(4):
                            urep_s = jpool.tile([P, 4 * F1], FP16, tag="ur")
                            nj = 4 if q < 3 else 3
                            nc.sync.dma_start(
                                urep_s[:, 0:nj * F1],
                                Urep[:, 4 * q * F1:(4 * q + nj) * F1])
                            for jj in range(nj):
                                j = 4 * q + jj
                                prod = jpool.tile([P, NB, F1], FP16,
                                                  tag="pr")
                                nc.vector.tensor_tensor(
                                    out=prod[:], in0=x1_all[:],
                                    in1=urep_s[:, jj * F1:(jj + 1) * F1]
                                        .unsqueeze(1)
                                        .broadcast_to([P, NB, F1]),
                                    op=OP.mult)
                                nc.vector.tensor_reduce(
                                    out=t2tmp[:, :, j], in_=prod[:],
                                    axis=AX.X, op=OP.add)
                        t2sb = mpool.tile([P, NB, T2W], FP16, tag="t2s")
                        nc.vector.tensor_copy(t2sb[:], t2tmp[:])
                        nc.sync.dma_start(
                            t2_loc[:, 0:T2W]
                                .rearrange("(b p) w -> p b w", p=P),
                            t2sb[:])

                if upto < 4:
                    continue
                nc.gpsimd.collective_compute(
                    "AllGather", OP.bypass, ins=[t2_loc.opt()],
                    outs=[t2_full.opt()], replica_groups=rg)

                # ---------------- layer-2 message passing (flat)
                with tc.tile_pool(name="l2", bufs=1) as lpool:
                    g2 = lpool.tile([P, nblk, KU, P], FP16, tag="g2")
                    for B in range(nblk):
                        nc.gpsimd.dma_gather(
                            out_ap=g2[:, B],
                            in_ap=t2_full[:],
                            idxs_ap=idx2_s[:, B * 64:(B + 1) * 64],
                            num_idxs=KU * P, num_idxs_reg=KU * P,
                            elem_size=P)
                    w2 = lpool.tile([P, nblk, KU, H2], F32, tag="w2")
                    nc.vector.tensor_tensor(
                        out=w2[:, 0:NB], in0=g2[:, 0:NB, :, H2:2 * H2],
                        in1=g2[:, 0:NB, 0:1, 2 * H2:3 * H2]
                            .broadcast_to([P, NB, KU, H2]), op=OP.add)
                    for t, (m, _j) in enumerate(exlist):
                        B = NB + t
                        nc.vector.tensor_tensor(
                            out=w2[:, B:B + 1],
                            in0=g2[:, B:B + 1, :, H2:2 * H2],
                            in1=g2[:, m:m + 1, 0:1, 2 * H2:3 * H2]
                                .broadcast_to([P, 1, KU, H2]), op=OP.add)
                    nc.vector.tensor_tensor(
                        out=w2, in0=w2,
                        in1=mask_s[:].rearrange("p (b k) -> p b k", k=KU)
                            .unsqueeze(3).broadcast_to([P, nblk, KU, H2]),
                        op=OP.add)
                    nc.vector.scalar_tensor_tensor(
                        out=w2[:], in0=w2[:], scalar=0.2,
                        in1=w2[:], op0=OP.mult, op1=OP.max)
                    w2h = lpool.tile([P, nblk, KU, H2], FP16, tag="w2h")
                    nc.scalar.activation(w2h[:], w2[:], AF.Exp)
                    den2 = mpool.tile([P, nblk, H2], F32, tag="dn2")
                    nc.vector.tensor_reduce(
                        out=den2[:], in_=w2h[:].transpose([0, 1, 3, 2]),
                        axis=AX.X, op=OP.add)
                    nc.vector.tensor_tensor(
                        out=g2[:, :, :, 0:H2], in0=g2[:, :, :, 0:H2],
                        in1=w2h[:], op=OP.mult)
                    num2 = mpool.tile([P, nblk, H2], F32, tag="nm2")
                    nc.vector.tensor_reduce(
                        out=num2[:],
                        in_=g2[:, :, :, 0:H2].transpose([0, 1, 3, 2]),
                        axis=AX.X, op=OP.add)
                    for (e0, e1, m0) in merges:
                        w_ = e1 - e0
                        nc.vector.tensor_tensor(
                            out=den2[:, m0:m0 + w_], in0=den2[:, m0:m0 + w_],
                            in1=den2[:, e0:e1], op=OP.add)
                        nc.vector.tensor_tensor(
                            out=num2[:, m0:m0 + w_], in0=num2[:, m0:m0 + w_],
                            in1=num2[:, e0:e1], op=OP.add)
                    x2 = mpool.tile([P, NB, H2], F32, tag="x2")
                    nc.vector.reciprocal(den2[:, 0:NB], den2[:, 0:NB])
                    nc.vector.tensor_tensor(
                        out=x2[:], in0=num2[:, 0:NB], in1=den2[:, 0:NB],
                        op=OP.mult)
                    sa = mpool.tile([P, NB], F32, tag="sa")
                    nc.vector.tensor_reduce(
                        out=sa[:], in_=x2[:], axis=AX.X, op=OP.add)
                    nc.sync.dma_start(
                        s_loc[:].rearrange("(b p) o -> p (b o)", p=P),
                        sa[:])

                nc.gpsimd.collective_compute(
                    "AllGather", OP.bypass, ins=[s_loc.opt()],
                    outs=[s_full.opt()], replica_groups=rg)

                if upto < 6:
                    continue
                # ---------------- finale: bond scores + softmax
                gl = mpool.tile([2 * N_BONDS, 1], F32, tag="gl")
                nc.gpsimd.indirect_dma_start(
                    out=gl[:], out_offset=None, in_=s_full[:],
                    in_offset=IndirectOffsetOnAxis(ap=lr_s[:, 0:1], axis=0))
                sc = mpool.tile([1, N_BONDS], F32, tag="sc")
                scb = mpool.tile([1, 2 * N_BONDS], F32, tag="scb")
                with tc.tile_pool(name="psf", bufs=1,
                                  space="PSUM") as pfpool:
                    ps_t = pfpool.tile([P, 512], F32, tag="pst")
                    nc.tensor.transpose(
                        out=ps_t[0:1, 0:P], in_=gl[:],
                        identity=ident_s[:])
                    nc.scalar.activation(scb[:], ps_t[0:1, 0:2 * N_BONDS],
                                         AF.Copy)
                nc.vector.tensor_tensor(
                    out=sc[:], in0=scb[:, 0:N_BONDS],
                    in1=scb[:, N_BONDS:2 * N_BONDS], op=OP.add)
                es = mpool.tile([1, N_BONDS], F32, tag="es")
                nc.scalar.activation(es[:], sc[:], AF.Exp)
                ssum = mpool.tile([1, 1], F32, tag="ss")
                nc.vector.tensor_reduce(
                    out=ssum[:], in_=es[:], axis=AX.X, op=OP.add)
                ys = mpool.tile([1, N_BONDS], F32, tag="ys")
                nc.vector.reciprocal(ssum[:], ssum[:])
                nc.vector.tensor_tensor(
                    out=ys[:], in0=es[:],
                    in1=ssum[:].to_broadcast([1, N_BONDS]), op=OP.mult)
                nc.sync.dma_start(y.ap().unsqueeze(0), ys[:])

    nc.compile()
    return nc


_PROGRAM_CACHE: dict = {}


def kernel(**inputs) -> np.ndarray:
    prep = _prep(np.asarray(inputs["edge_index"], np.int64))
    exlist, nblk = prep[0], prep[1]
    key = (tuple(exlist), nblk)
    if key not in _PROGRAM_CACHE:
        _PROGRAM_CACHE[key] = _build_program(exlist, nblk)
    nc = _PROGRAM_CACHE[key]
    w1ext, urep = _fold_weights(inputs)
    in_maps = [_make_core_inputs(inputs, prep, c, urep, w1ext)
               for c in range(N_CORES)]
    res = run_bass_kernel_spmd(nc, in_maps, core_ids=list(range(N_CORES)))
    return res.results[0]["y"]


if __name__ == "__main__":
    import jax

    import reference

    with jax.default_device(jax.devices("cpu")[0]):
        inputs = {k: np.asarray(v) for k, v in reference.setup_inputs().items()}
        expected = np.asarray(reference.reference(**reference.setup_inputs()))
    actual = kernel(**inputs)
    rel = np.abs(actual - expected).max() / np.abs(expected).max()
    print("Relative error:", rel)
